# revision 16
# baseline (speedup 1.0000x reference)
"""CrossLayerTranscoder Trainium2 kernel.

Shards the d_transcoder (feature) axis across 8 NeuronCores (768 features
per layer per core).  Each core encodes its feature shard for all 6 layers
(acts kept feature-major on-chip), then decodes partial reconstructions for
every layer j accumulating over source layers i <= j.  The feature-shard
all-reduce is done on the host when unsharding (partials summed + b_dec).

All device tensors are bf16 (weights pre-converted on the host) so the
kernel streams half the HBM bytes of the f32 version; PSUM accumulation
stays f32 and the per-core output partials are written bf16 and summed in
f32 on the host.  Input loads ride the Sync-engine HWDGE queue as few big
contiguous transfers; output writes ride the Scalar-engine HWDGE queue so
they never head-of-line-block the weight stream.
"""

import numpy as np

import concourse.bass as bass
import concourse.mybir as mybir
from concourse.bass import ts
from concourse.tile import TileContext
from concourse.bass_utils import run_bass_kernel_spmd

L = 6            # layers
T = 128          # tokens
D = 768          # d_model
DT = 6144        # d_transcoder
N_CORES = 8
F = DT // N_CORES   # features per layer per core = 768
KD = D // 128       # d_model chunks of 128 = 6
KF = F // 128       # feature chunks of 128 = 6
# decode pairs in i-outer order (only upper triangle j >= i is nonzero):
# row i becomes computable right after layer i is encoded, which spreads
# decode matmuls into the encode phase instead of batching them at the end
PAIRS = [(i, j) for i in range(L) for j in range(i, L)]
PAIR_IDX = {p: n for n, p in enumerate(PAIRS)}

F32 = mybir.dt.float32
BF16 = mybir.dt.bfloat16
FP8E3 = mybir.dt.float8e3
# W_dec is stored fp8 e3m4 (4 mantissa bits ~ 1.3% RMS on gaussian weights,
# measured 1.37% end-to-end vs the 2e-2 gate).  Power-of-two scale keeps the
# descale exact; it is folded into the PSUM->SBUF copy.
WD_SCALE = 128.0


def _split_multiwaits(nc):
    """This container's walrus rejects >1 sync-wait per instruction; split
    extra waits onto same-engine NOPs inserted immediately before."""
    for fn in nc.m.functions:
        for bb in fn.blocks:
            new = []
            for ins in bb.instructions:
                si = ins.sync_info
                if si is not None and si.on_wait and len(si.on_wait) > 1:
                    waits = list(si.on_wait)
                    for w in waits[:-1]:
                        nop = mybir.InstNoOp(
                            name=nc.get_next_instruction_name(),
                            engine=ins.engine,
                            ins=[],
                            outs=[],
                            sync_info=mybir.SyncInfo(on_wait=[w], on_update=[]),
                        )
                        new.append(nop)
                    ins.sync_info = mybir.SyncInfo(
                        on_wait=[waits[-1]], on_update=list(si.on_update or [])
                    )
                new.append(ins)
            bb.instructions = new


def _build_nc():
    nc = bass.Bass()
    # partition-major host layouts: every DMA is 128 partitions x a single
    # contiguous per-partition run
    xt_d = nc.dram_tensor("xt", [128, L, KD, T], BF16, kind="ExternalInput")
    we_d = nc.dram_tensor("we", [L, 128, KD, F], BF16, kind="ExternalInput")
    wd_d = nc.dram_tensor("wd", [128, len(PAIRS), KF, D], FP8E3, kind="ExternalInput")
    be_d = nc.dram_tensor("be", [128, L, KF], F32, kind="ExternalInput")
    out_d = nc.dram_tensor("out", [L, 128, D], BF16, kind="ExternalOutput")

    with TileContext(nc) as tc:
        with (
            tc.tile_pool(name="const", bufs=1) as cpool,
            tc.tile_pool(name="w", bufs=14) as wpool,
            tc.tile_pool(name="o", bufs=3) as opool,
            tc.tile_pool(name="pse", bufs=2, space="PSUM") as pse,
            tc.tile_pool(name="psd", bufs=6, space="PSUM") as psd,
        ):
            BE = cpool.tile([128, L, KF], F32, tag="be")
            A = cpool.tile([128, L, KF, T], BF16, tag="acts")
            # per-j reconstruction accumulators live in SBUF (PSUM can't hold
            # six j-accumulators at once); DVE folds each pair's PSUM product
            # in with the fp8 descale applied
            ACC = cpool.tile([128, L, D], F32, tag="acc")
            # x loads ride the Scalar-engine queue so they land in parallel
            # with we0 on the Sync queue — the first matmul needs only
            # X[0] + we[0], not the whole x tensor
            Xs = []
            for l in range(L):
                Xl = cpool.tile([128, KD, T], BF16, tag=f"x{l}")
                Xs.append(Xl)
                nc.scalar.dma_start(out=Xl[:], in_=xt_d[:, l])
                if l == 0:
                    nc.scalar.dma_start(out=BE[:], in_=be_d[:])

            # wd streams in 2-pair slabs (9216B contiguous per partition run)
            SLAB = 2
            slab_tiles = {}

            def wd_pair(n):
                s = n // SLAB
                if s not in slab_tiles:
                    n0 = s * SLAB
                    k = min(SLAB, len(PAIRS) - n0)
                    t = wpool.tile([128, k, KF, D], FP8E3, tag="w")
                    nc.sync.dma_start(out=t[:], in_=wd_d[:, n0 : n0 + k])
                    slab_tiles[s] = t
                return slab_tiles[s], n % SLAB

            HALves = ((0, slice(0, 384)), (1, slice(384, 768)))

            for l in range(L):
                # ---- encode layer l: acts[f, t] = relu(We^T-chunks @ x^T + be)
                we = wpool.tile([128, KD, F], BF16, tag="w")
                nc.sync.dma_start(out=we[:], in_=we_d[l])
                for ft in range(KF):
                    ps = pse.tile([128, T], F32, tag="pse")
                    for kd in range(KD):
                        nc.tensor.matmul(
                            ps[:],
                            we[:, kd, ts(ft, 128)],
                            Xs[l][:, kd, :],
                            start=(kd == 0),
                            stop=(kd == KD - 1),
                        )
                    # relu(ps + b_enc) on DVE — keeps ScalarE free for the
                    # output-DMA trigger queue
                    nc.vector.tensor_scalar(
                        out=A[:, l, ft, :],
                        in0=ps[:],
                        scalar1=BE[:, l, ts(ft, 1)],
                        scalar2=0.0,
                        op0=mybir.AluOpType.add,
                        op1=mybir.AluOpType.max,
                    )

                # ---- decode row l: ACC[j] += acts_l^T-chunks @ W_dec[l,j]
                for j in range(l, L):
                    n = PAIR_IDX[(l, j)]
                    wd, sl = wd_pair(n)
                    ps0 = psd.tile([128, 384], F32, tag="psd")
                    ps1 = psd.tile([128, 384], F32, tag="psd")
                    pss = (ps0, ps1)
                    if n == len(PAIRS) - 1:
                        # d-half-outer on the final pair: ps0 closes 6 matmuls
                        # early so its ACC-add/copy/output-DMA overlap the
                        # ps1 tail
                        for h, dsl in HALves:
                            for kf in range(KF):
                                nc.tensor.matmul(
                                    pss[h][:], A[:, l, kf, :], wd[:, sl, kf, dsl],
                                    start=(kf == 0),
                                    stop=(kf == KF - 1),
                                )
                    else:
                        # kf outer so each acts chunk is LDWEIGHTS'ed once
                        for kf in range(KF):
                            for h, dsl in HALves:
                                nc.tensor.matmul(
                                    pss[h][:], A[:, l, kf, :], wd[:, sl, kf, dsl],
                                    start=(kf == 0),
                                    stop=(kf == KF - 1),
                                )
                    for h, dsl in HALves:
                        if l == 0:
                            nc.vector.tensor_scalar_mul(
                                out=ACC[:, j, dsl], in0=pss[h][:],
                                scalar1=1.0 / WD_SCALE,
                            )
                        else:
                            nc.vector.scalar_tensor_tensor(
                                out=ACC[:, j, dsl], in0=pss[h][:],
                                scalar=1.0 / WD_SCALE, in1=ACC[:, j, dsl],
                                op0=mybir.AluOpType.mult,
                                op1=mybir.AluOpType.add,
                            )
                        if l == j:
                            # ACC[j] complete — cast to bf16 and ship
                            OUTj = opool.tile([128, 384], BF16, tag="out")
                            nc.vector.tensor_copy(out=OUTj[:], in_=ACC[:, j, dsl])
                            nc.scalar.dma_start(out=out_d[j, :, dsl], in_=OUTj[:])

    _split_multiwaits(nc)
    return nc


_NC_CACHE = {}


def _get_nc():
    if "nc" not in _NC_CACHE:
        _NC_CACHE["nc"] = _build_nc()
    return _NC_CACHE["nc"]


def _np_bf16():
    import ml_dtypes

    return np.dtype(ml_dtypes.bfloat16)


def _shard_inputs(x, W_enc, b_enc):
    """Host-side pre-swizzle into per-core DMA-friendly layouts."""
    npdt = _np_bf16()
    # xt[p, l, kd, t] = x[l, t, kd*128+p] — same on every core
    xt = np.ascontiguousarray(
        x.transpose(2, 0, 1).reshape(KD, 128, L, T).transpose(1, 2, 0, 3)
    ).astype(npdt)
    in_maps = []
    for c in range(N_CORES):
        fs = c * F
        w = W_enc[:, fs : fs + F, :]  # [L, F, D]
        # we[l, p, kd, f] = W_enc[l, fs+f, kd*128+p]
        we = np.ascontiguousarray(
            w.transpose(0, 2, 1).reshape(L, KD, 128, F).transpose(0, 2, 1, 3)
        ).astype(npdt)
        be = np.ascontiguousarray(
            b_enc[:, fs : fs + F].reshape(L, KF, 128).transpose(2, 0, 1)
        ).astype(np.float32)
        in_maps.append({"xt": xt, "we": we, "be": be})
    return in_maps


def _shard_wdec(W_dec):
    import ml_dtypes

    npdt = np.dtype(ml_dtypes.float8_e3m4)
    shards = []
    for c in range(N_CORES):
        fs = c * F
        # wd[p, n, kf, d] = W_dec[i_n, j_n, fs + kf*128 + p, d] * WD_SCALE
        wd = np.empty((128, len(PAIRS), KF, D), dtype=npdt)
        for n, (i, j) in enumerate(PAIRS):
            blk = W_dec[i, j, fs : fs + F, :] * WD_SCALE  # [F, D]
            wd[:, n] = blk.reshape(KF, 128, D).transpose(1, 0, 2).astype(npdt)
        shards.append(wd)
    return shards


def kernel(x, W_enc, b_enc, b_dec, W_dec, dec_mask=None, **_unused):
    x = np.asarray(x, dtype=np.float32)
    W_enc = np.asarray(W_enc, dtype=np.float32)
    b_enc = np.asarray(b_enc, dtype=np.float32)
    b_dec = np.asarray(b_dec, dtype=np.float32)
    W_dec = np.asarray(W_dec, dtype=np.float32)

    nc = _get_nc()

    in_maps = _shard_inputs(x, W_enc, b_enc)
    wd_shards = _shard_wdec(W_dec)
    for c in range(N_CORES):
        in_maps[c]["wd"] = wd_shards[c]

    res = run_bass_kernel_spmd(nc, in_maps, core_ids=list(range(N_CORES)))

    # host-side all-reduce over feature shards + decoder bias
    recon = np.zeros((L, T, D), dtype=np.float32)
    for c in range(N_CORES):
        recon += res.results[c]["out"].astype(np.float32)
    recon += b_dec[:, None, :]
    return recon


# revision 17
# speedup vs baseline: 1.0640x; 1.0640x over previous
"""CrossLayerTranscoder Trainium2 kernel.

Shards the d_transcoder (feature) axis across 8 NeuronCores (768 features
per layer per core).  Each core encodes its feature shard for all 6 layers
(acts kept feature-major on-chip), then decodes partial reconstructions for
every layer j accumulating over source layers i <= j.  The feature-shard
all-reduce is done on the host when unsharding (partials summed + b_dec).

x/W_enc/acts are bf16 and W_dec is fp8 e3m4 (host-converted, power-of-two
scale, descale folded into the PSUM->SBUF accumulate), cutting HBM traffic
~3.2x vs f32.  PSUM accumulation stays f32; per-core output partials are
written bf16 and summed in f32 on the host.  Decode runs i-major so row i's
matmuls start right after layer i encodes (PE work overlaps the weight
stream), with per-j accumulators in SBUF via DVE adds.  Weight loads ride
the Sync-engine HWDGE queue; x/bias loads and output writes ride the
Scalar-engine queue so neither blocks the weight stream.
"""

import numpy as np

import concourse.bass as bass
import concourse.mybir as mybir
from concourse.bass import ts
from concourse.tile import TileContext
from concourse.bass_utils import run_bass_kernel_spmd

L = 6            # layers
T = 128          # tokens
D = 768          # d_model
DT = 6144        # d_transcoder
N_CORES = 8
F = DT // N_CORES   # features per layer per core = 768
KD = D // 128       # d_model chunks of 128 = 6
KF = F // 128       # feature chunks of 128 = 6
# decode pairs in i-outer order (only upper triangle j >= i is nonzero):
# row i becomes computable right after layer i is encoded, which spreads
# decode matmuls into the encode phase instead of batching them at the end
PAIRS = [(i, j) for i in range(L) for j in range(i, L)]
PAIR_IDX = {p: n for n, p in enumerate(PAIRS)}

F32 = mybir.dt.float32
BF16 = mybir.dt.bfloat16
FP8E3 = mybir.dt.float8e3
# W_dec is stored fp8 e3m4 (4 mantissa bits ~ 1.3% RMS on gaussian weights,
# measured 1.37% end-to-end vs the 2e-2 gate).  Power-of-two scale keeps the
# descale exact; it is folded into the PSUM->SBUF copy.
WD_SCALE = 128.0


def _split_multiwaits(nc):
    """This container's walrus rejects >1 sync-wait per instruction; split
    extra waits onto same-engine NOPs inserted immediately before."""
    for fn in nc.m.functions:
        for bb in fn.blocks:
            new = []
            for ins in bb.instructions:
                si = ins.sync_info
                if si is not None and si.on_wait and len(si.on_wait) > 1:
                    waits = list(si.on_wait)
                    for w in waits[:-1]:
                        nop = mybir.InstNoOp(
                            name=nc.get_next_instruction_name(),
                            engine=ins.engine,
                            ins=[],
                            outs=[],
                            sync_info=mybir.SyncInfo(on_wait=[w], on_update=[]),
                        )
                        new.append(nop)
                    ins.sync_info = mybir.SyncInfo(
                        on_wait=[waits[-1]], on_update=list(si.on_update or [])
                    )
                new.append(ins)
            bb.instructions = new


def _build_nc():
    nc = bass.Bass()
    # partition-major host layouts: every DMA is 128 partitions x a single
    # contiguous per-partition run
    xt_d = nc.dram_tensor("xt", [128, L, KD, T], BF16, kind="ExternalInput")
    we_d = nc.dram_tensor("we", [L, 128, KD, F], BF16, kind="ExternalInput")
    wd_d = nc.dram_tensor("wd", [128, len(PAIRS), KF, D], FP8E3, kind="ExternalInput")
    be_d = nc.dram_tensor("be", [128, L, KF], F32, kind="ExternalInput")
    out_d = nc.dram_tensor("out", [L, 128, D], BF16, kind="ExternalOutput")

    with TileContext(nc) as tc:
        with (
            tc.tile_pool(name="const", bufs=1) as cpool,
            tc.tile_pool(name="w", bufs=10) as wpool,
            tc.tile_pool(name="o", bufs=3) as opool,
            tc.tile_pool(name="pse", bufs=2, space="PSUM") as pse,
            tc.tile_pool(name="psd", bufs=4, space="PSUM") as psd,
        ):
            BE = cpool.tile([128, L, KF], F32, tag="be")
            A = cpool.tile([128, L, KF, T], BF16, tag="acts")
            # per-j reconstruction accumulators live in SBUF (PSUM can't hold
            # six j-accumulators at once); DVE folds each pair's PSUM product
            # in with the fp8 descale applied
            ACC = cpool.tile([128, L, D], F32, tag="acc")
            # x loads ride the Scalar-engine queue so they land in parallel
            # with we0 on the Sync queue — the first matmul needs only
            # X[0] + we[0], not the whole x tensor
            Xs = []
            for l in range(L):
                Xl = cpool.tile([128, KD, T], BF16, tag=f"x{l}")
                Xs.append(Xl)
                nc.scalar.dma_start(out=Xl[:], in_=xt_d[:, l])
                if l == 0:
                    nc.scalar.dma_start(out=BE[:], in_=be_d[:])

            # wd streams in 2-pair slabs (9216B contiguous per partition run)
            SLAB = 2
            slab_tiles = {}

            def wd_pair(n):
                s = n // SLAB
                if s not in slab_tiles:
                    n0 = s * SLAB
                    k = min(SLAB, len(PAIRS) - n0)
                    t = wpool.tile([128, k, KF, D], FP8E3, tag="w")
                    nc.sync.dma_start(out=t[:], in_=wd_d[:, n0 : n0 + k])
                    slab_tiles[s] = t
                return slab_tiles[s], n % SLAB

            HALves = ((0, slice(0, 384)), (1, slice(384, 768)))

            for l in range(L):
                # ---- encode layer l: acts[f, t] = relu(We^T-chunks @ x^T + be)
                we = wpool.tile([128, KD, F], BF16, tag="w")
                nc.sync.dma_start(out=we[:], in_=we_d[l])
                for ft in range(KF):
                    ps = pse.tile([128, T], F32, tag="pse")
                    for kd in range(KD):
                        nc.tensor.matmul(
                            ps[:],
                            we[:, kd, ts(ft, 128)],
                            Xs[l][:, kd, :],
                            start=(kd == 0),
                            stop=(kd == KD - 1),
                        )
                    # relu(ps + b_enc) on DVE — keeps ScalarE free for the
                    # output-DMA trigger queue
                    nc.vector.tensor_scalar(
                        out=A[:, l, ft, :],
                        in0=ps[:],
                        scalar1=BE[:, l, ts(ft, 1)],
                        scalar2=0.0,
                        op0=mybir.AluOpType.add,
                        op1=mybir.AluOpType.max,
                    )

                # ---- decode row l: ACC[j] += acts_l^T-chunks @ W_dec[l,j]
                for j in range(l, L):
                    n = PAIR_IDX[(l, j)]
                    wd, sl = wd_pair(n)
                    ps0 = psd.tile([128, 384], F32, tag="psd")
                    ps1 = psd.tile([128, 384], F32, tag="psd")
                    pss = (ps0, ps1)
                    if n == len(PAIRS) - 1:
                        # d-half-outer on the final pair: ps0 closes 6 matmuls
                        # early so its ACC-add/copy/output-DMA overlap the
                        # ps1 tail
                        for h, dsl in HALves:
                            for kf in range(KF):
                                nc.tensor.matmul(
                                    pss[h][:], A[:, l, kf, :], wd[:, sl, kf, dsl],
                                    start=(kf == 0),
                                    stop=(kf == KF - 1),
                                )
                    else:
                        # kf outer so each acts chunk is LDWEIGHTS'ed once
                        for kf in range(KF):
                            for h, dsl in HALves:
                                nc.tensor.matmul(
                                    pss[h][:], A[:, l, kf, :], wd[:, sl, kf, dsl],
                                    start=(kf == 0),
                                    stop=(kf == KF - 1),
                                )
                    for h, dsl in HALves:
                        if l == 0:
                            nc.vector.tensor_scalar_mul(
                                out=ACC[:, j, dsl], in0=pss[h][:],
                                scalar1=1.0 / WD_SCALE,
                            )
                        else:
                            nc.vector.scalar_tensor_tensor(
                                out=ACC[:, j, dsl], in0=pss[h][:],
                                scalar=1.0 / WD_SCALE, in1=ACC[:, j, dsl],
                                op0=mybir.AluOpType.mult,
                                op1=mybir.AluOpType.add,
                            )
                        if l == j:
                            # ACC[j] complete — cast to bf16 and ship
                            OUTj = opool.tile([128, 384], BF16, tag="out")
                            nc.vector.tensor_copy(out=OUTj[:], in_=ACC[:, j, dsl])
                            nc.scalar.dma_start(out=out_d[j, :, dsl], in_=OUTj[:])

    _split_multiwaits(nc)
    return nc


_NC_CACHE = {}


def _get_nc():
    if "nc" not in _NC_CACHE:
        _NC_CACHE["nc"] = _build_nc()
    return _NC_CACHE["nc"]


def _np_bf16():
    import ml_dtypes

    return np.dtype(ml_dtypes.bfloat16)


def _shard_inputs(x, W_enc, b_enc):
    """Host-side pre-swizzle into per-core DMA-friendly layouts."""
    npdt = _np_bf16()
    # xt[p, l, kd, t] = x[l, t, kd*128+p] — same on every core
    xt = np.ascontiguousarray(
        x.transpose(2, 0, 1).reshape(KD, 128, L, T).transpose(1, 2, 0, 3)
    ).astype(npdt)
    in_maps = []
    for c in range(N_CORES):
        fs = c * F
        w = W_enc[:, fs : fs + F, :]  # [L, F, D]
        # we[l, p, kd, f] = W_enc[l, fs+f, kd*128+p]
        we = np.ascontiguousarray(
            w.transpose(0, 2, 1).reshape(L, KD, 128, F).transpose(0, 2, 1, 3)
        ).astype(npdt)
        be = np.ascontiguousarray(
            b_enc[:, fs : fs + F].reshape(L, KF, 128).transpose(2, 0, 1)
        ).astype(np.float32)
        in_maps.append({"xt": xt, "we": we, "be": be})
    return in_maps


def _shard_wdec(W_dec):
    import ml_dtypes

    npdt = np.dtype(ml_dtypes.float8_e3m4)
    shards = []
    for c in range(N_CORES):
        fs = c * F
        # wd[p, n, kf, d] = W_dec[i_n, j_n, fs + kf*128 + p, d] * WD_SCALE
        wd = np.empty((128, len(PAIRS), KF, D), dtype=npdt)
        for n, (i, j) in enumerate(PAIRS):
            blk = W_dec[i, j, fs : fs + F, :] * WD_SCALE  # [F, D]
            wd[:, n] = blk.reshape(KF, 128, D).transpose(1, 0, 2).astype(npdt)
        shards.append(wd)
    return shards


def kernel(x, W_enc, b_enc, b_dec, W_dec, dec_mask=None, **_unused):
    x = np.asarray(x, dtype=np.float32)
    W_enc = np.asarray(W_enc, dtype=np.float32)
    b_enc = np.asarray(b_enc, dtype=np.float32)
    b_dec = np.asarray(b_dec, dtype=np.float32)
    W_dec = np.asarray(W_dec, dtype=np.float32)

    nc = _get_nc()

    in_maps = _shard_inputs(x, W_enc, b_enc)
    wd_shards = _shard_wdec(W_dec)
    for c in range(N_CORES):
        in_maps[c]["wd"] = wd_shards[c]

    res = run_bass_kernel_spmd(nc, in_maps, core_ids=list(range(N_CORES)))

    # host-side all-reduce over feature shards + decoder bias
    recon = np.zeros((L, T, D), dtype=np.float32)
    for c in range(N_CORES):
        recon += res.results[c]["out"].astype(np.float32)
    recon += b_dec[:, None, :]
    return recon
